# revision 2
# baseline (speedup 1.0000x reference)
"""Self-contained Bass/Trainium2 kernel for single-head causal self-attention.

reference semantics (fp32):
  qkv = x @ Wqkv; q,k,v = split(qkv)
  att = softmax(causal(q k^T / sqrt(C)))
  y = (att @ v) @ Wproj

Sharding: 8 cores = 4 batches x 2 causally-balanced query-tile sets.
Program A (cores 0-3): q-tiles {0..7, 24..31} of its batch.
Program B (cores 4-7): q-tiles {8..23} of its batch.
Both process 72 key-chunks of attention work; each runs as its own NEFF
on a disjoint 4-device mesh, dispatched concurrently.

v2 layout: all matmul operands bf16 (halves PE weight-load time vs
f32r); V kept in SBUF (no DRAM round-trip); attention output computed
transposed (O^T = V^T-slices @ P^T) so the Wproj matmul needs no
output transposes and the softmax 1/l folds into the per-partition
PSUM eviction scale of y. Row sums l via a ones[128,128] stationary.
exp is biased by -1.5 (cancels in normalization) to center p in bf16.
The S(t+1) matmul chain is emitted before O(t) so the PE never waits
on the Act engine's exp.
"""

import sys

sys.path.insert(0, "/opt/trn_rl_repo")

import numpy as np

B, T, C = 4, 4096, 512
TQ = 2048               # q rows per core
N_CORES = 8
SCALE = 1.0 / np.sqrt(C)
MASKVAL = -1.0e10
EXP_BIAS = -1.5

GROUPS_A = [0, 4, 24, 28]    # group base tile (tiles a..a+3), program A
GROUPS_B = [8, 12, 16, 20]
KV_CHUNKS_A = 8              # 512-row x chunks needed for K/V
KV_CHUNKS_B = 6
Q_CHUNKS_A = [0, 1, 6, 7]    # x chunks holding the program's q rows
Q_CHUNKS_B = [2, 3, 4, 5]

_CACHE = {}


def _dmask_np():
    # [128, 4*512] additive masks for the 4 diagonal-offset variants.
    # Variant d, sub-tile k columns: k<d fully masked, k==d triangular
    # (valid where j' <= i'), k>d fully visible. Applied to S^T tiles
    # with keys on partitions, queries on the free dim.
    m = np.zeros((128, 4, 4, 128), dtype=np.float32)
    jj = np.arange(128)[:, None]
    ii = np.arange(128)[None, :]
    tri = np.where(jj <= ii, 0.0, MASKVAL).astype(np.float32)
    for d in range(4):
        for k in range(4):
            if k < d:
                m[:, d, k, :] = MASKVAL
            elif k == d:
                m[:, d, k, :] = tri
    return m.reshape(128, 4 * 512)


def _build(group_starts, kv_chunks, q_chunks):
    import ml_dtypes
    import concourse.mybir as mybir
    import concourse.tile as tile
    from concourse import bacc

    F32 = mybir.dt.float32
    BF16 = mybir.dt.bfloat16
    AF = mybir.ActivationFunctionType
    TKV = kv_chunks * 512
    NKEY = kv_chunks * 4         # 128-row key sub-chunks

    nc = bacc.Bacc("TRN2", target_bir_lowering=False, debug=False,
                   num_devices=4)

    x_in = nc.dram_tensor("x_in", [T, C], F32, kind="ExternalInput").ap()
    wqkv_in = nc.dram_tensor("wqkv", [C, 3 * C], F32, kind="ExternalInput").ap()
    wproj_in = nc.dram_tensor("wproj", [C, C], F32, kind="ExternalInput").ap()
    y_out = nc.dram_tensor("y", [TQ, C], F32, kind="ExternalOutput").ap()

    dmask_d = nc.inline_tensor(
        _dmask_np().astype(ml_dtypes.bfloat16), name="dmask").ap()
    ident_d = nc.inline_tensor(
        np.eye(128, dtype=ml_dtypes.bfloat16), name="ident").ap()

    with tile.TileContext(nc) as tc:
        with tc.tile_pool(name="persist", bufs=1) as pp:
            kT = pp.tile([128, 4, TKV], BF16)        # K^T  [c-chunk, j]
            qT = pp.tile([128, 4, TQ], BF16)         # Q^T  [c-chunk, i]
            v_sb = pp.tile([128, NKEY, C], BF16)     # V    [key-sub, c]
            wqkv_sb = pp.tile([128, 4, 3 * C], BF16)
            wproj_sb = pp.tile([128, 4, C], BF16)
            dm_sb = pp.tile([128, 4, 512], BF16)     # diagonal masks
            id_sb = pp.tile([128, 128], BF16)
            ones_sb = pp.tile([128, 128], BF16)
            bias_sb = pp.tile([128, 1], F32)

            nc.sync.dma_start(id_sb[:], ident_d[:])
            nc.sync.dma_start(dm_sb[:], dmask_d.rearrange("p (d n) -> p d n", d=4))
            nc.vector.memset(ones_sb[:], 1.0)
            nc.vector.memset(bias_sb[:], EXP_BIAS)
            with tc.tile_pool(name="wtmp", bufs=2) as wt:
                for c in range(4):
                    wq_raw = wt.tile([128, 3 * C], F32, tag="wqr")
                    nc.sync.dma_start(
                        wq_raw[:], wqkv_in[128 * c:128 * (c + 1), :])
                    nc.gpsimd.tensor_copy(wqkv_sb[:, c, :], wq_raw[:])
                wp_raw = wt.tile([128, 4, C], F32, tag="wpr")
                nc.sync.dma_start(wp_raw[:],
                                  wproj_in.rearrange("(k p) f -> p k f", p=128))
                nc.gpsimd.tensor_copy(wproj_sb[:], wp_raw[:])

            # ---------------- Phase 1: x^T, K^T, Q^T, V ----------------
            # Software-pipelined: transposes of chunk t overlap the
            # K/Q/V matmuls of chunk t-1 (PE never waits on DVE evicts).
            with tc.tile_pool(name="p1", bufs=3) as p1, \
                 tc.tile_pool(name="p1b", bufs=2) as p1b, \
                 tc.tile_pool(name="p1xt", bufs=2, space="PSUM") as p1xt, \
                 tc.tile_pool(name="p1kv", bufs=2, space="PSUM") as p1kv:

                x_f = {}
                x_b = {}
                xT = {}

                def stage_transpose(tch):
                    x_f[tch] = p1.tile([128, 4, 512], F32, tag="x",
                                       name=f"x_f{tch}")
                    nc.sync.dma_start(
                        x_f[tch][:],
                        x_in[512 * tch:512 * (tch + 1), :]
                        .rearrange("(n p) c -> p n c", p=128))
                    x_b[tch] = p1b.tile([128, 4, 512], BF16, tag="xb",
                                        name=f"x_b{tch}")
                    nc.gpsimd.tensor_copy(x_b[tch][:], x_f[tch][:])
                    xT[tch] = p1b.tile([128, 4, 512], BF16, tag="xT",
                                       name=f"xT{tch}")
                    for n in range(4):
                        ps_xt = p1xt.tile([128, 512], BF16, tag="xt")
                        for c in range(4):
                            nc.tensor.transpose(
                                ps_xt[:, 128 * c:128 * (c + 1)],
                                x_b[tch][:, n, 128 * c:128 * (c + 1)],
                                id_sb[:])
                        nc.vector.tensor_copy(
                            xT[tch][:, :, 128 * n:128 * (n + 1)],
                            ps_xt[:].rearrange("p (c q) -> p c q", c=4))

                def stage_kqv(tch):
                    xt = xT[tch]
                    # K^T tiles (evict on Act: idle during phase 1)
                    for f in range(4):
                        ps_k = p1kv.tile([128, 512], F32, tag="kv")
                        for c in range(4):
                            nc.tensor.matmul(
                                ps_k[:],
                                wqkv_sb[:, c, C + 128 * f:C + 128 * (f + 1)],
                                xt[:, c, :],
                                start=(c == 0), stop=(c == 3))
                        nc.scalar.copy(
                            kT[:, f, 512 * tch:512 * (tch + 1)], ps_k[:])
                    # V tiles -> SBUF (keys on partitions, natural layout)
                    for n in range(4):
                        ps_v = p1kv.tile([128, 512], F32, tag="kv")
                        for c in range(4):
                            nc.tensor.matmul(
                                ps_v[:],
                                xt[:, c, 128 * n:128 * (n + 1)],
                                wqkv_sb[:, c, 2 * C:3 * C],
                                start=(c == 0), stop=(c == 3))
                        nc.vector.tensor_copy(v_sb[:, 4 * tch + n, :], ps_v[:])
                    # Q^T tiles for this program's q rows
                    if tch in q_chunks:
                        slot = q_chunks.index(tch)
                        for f in range(4):
                            ps_q = p1kv.tile([128, 512], F32, tag="kv")
                            for c in range(4):
                                nc.tensor.matmul(
                                    ps_q[:],
                                    wqkv_sb[:, c, 128 * f:128 * (f + 1)],
                                    xt[:, c, :],
                                    start=(c == 0), stop=(c == 3))
                            nc.scalar.copy(
                                qT[:, f, 512 * slot:512 * (slot + 1)],
                                ps_q[:])

                for tch in range(kv_chunks + 1):
                    if tch < kv_chunks:
                        stage_transpose(tch)
                    if tch > 0:
                        stage_kqv(tch - 1)
                        del x_f[tch - 1], x_b[tch - 1]

            # ---------------- Phase 2: attention + projection ----------
            with tc.tile_pool(name="p2", bufs=1) as p2, \
                 tc.tile_pool(name="psS", bufs=2, space="PSUM") as psS, \
                 tc.tile_pool(name="psO", bufs=1, space="PSUM") as psO, \
                 tc.tile_pool(name="psl", bufs=1, space="PSUM") as psl:
                for g, a in enumerate(group_starts):
                    trip = a + 4
                    o_ps = [psO.tile([128, 512], F32, tag=f"o{cs}",
                                     name=f"o_ps{g}_{cs}") for cs in range(4)]
                    l_ps = psl.tile([128, 512], F32, tag="l", name=f"l{g}")

                    def s_chunk(t):
                        s_ps = psS.tile([128, 512], F32, tag="s",
                                        name=f"s{g}_{t}")
                        for c in range(4):
                            nc.tensor.matmul(
                                s_ps[:],
                                kT[:, c, 128 * t:128 * (t + 1)],
                                qT[:, c, 512 * g:512 * (g + 1)],
                                start=(c == 0), stop=(c == 3))
                        d = t - a
                        if d >= 0:
                            nc.vector.tensor_add(s_ps[:], s_ps[:],
                                                 dm_sb[:, d, :])
                        pT = p2.tile([128, 512], BF16, tag="pT", bufs=3)
                        nc.scalar.activation(pT[:], s_ps[:], AF.Exp,
                                             bias=bias_sb[:], scale=SCALE)
                        return pT

                    def o_chunk(t, pT):
                        first, last = (t == 0), (t == trip - 1)
                        for cs in range(4):
                            nc.tensor.matmul(
                                o_ps[cs][:],
                                v_sb[:, t, 128 * cs:128 * (cs + 1)],
                                pT[:],
                                start=first, stop=last,
                                skip_group_check=True)
                        nc.tensor.matmul(
                            l_ps[:], ones_sb[:], pT[:],
                            start=first, stop=last,
                            skip_group_check=True)

                    # pipeline: S(t+1) is emitted before O(t) so the PE
                    # works through the next score tile while Act runs exp
                    pT_prev = s_chunk(0)
                    for t in range(1, trip):
                        pT_cur = s_chunk(t)
                        o_chunk(t - 1, pT_prev)
                        pT_prev = pT_cur
                    o_chunk(trip - 1, pT_prev)

                    # normalization scalars: r = 1/l per query row
                    l_sb = p2.tile([128, 512], BF16, tag="lsb", bufs=2)
                    nc.vector.tensor_copy(l_sb[:], l_ps[:])
                    lT_ps = psl.tile([128, 4, 128], BF16, tag="l",
                                     name=f"lT{g}")
                    for ts in range(4):
                        nc.tensor.transpose(
                            lT_ps[:, ts, :],
                            l_sb[:, 128 * ts:128 * (ts + 1)], id_sb[:])
                    r_sb = p2.tile([128, 4], F32, tag="r", bufs=2)
                    for ts in range(4):
                        nc.vector.reciprocal(r_sb[:, ts:ts + 1],
                                             lT_ps[:, ts, 0:1])

                    # attention output (transposed) -> bf16
                    oT_sb = p2.tile([128, 4, 512], BF16, tag="oT", bufs=2)
                    for cs in range(4):
                        nc.vector.tensor_copy(oT_sb[:, cs, :], o_ps[cs][:])

                    # projection: y[tok, f] accumulated over c-subtiles,
                    # scaled by r at eviction (each ts reuses an o bank)
                    for ts in range(4):
                        y_ps = psO.tile([128, 512], F32, tag=f"o{ts}",
                                        name=f"y{g}_{ts}")
                        for c in range(4):
                            nc.tensor.matmul(
                                y_ps[:],
                                oT_sb[:, c, 128 * ts:128 * (ts + 1)],
                                wproj_sb[:, c, :],
                                start=(c == 0), stop=(c == 3))
                        y_sb = p2.tile([128, 512], F32, tag="ysb", bufs=2)
                        nc.vector.tensor_scalar_mul(y_sb[:], y_ps[:],
                                                    r_sb[:, ts:ts + 1])
                        r0 = 128 * (4 * g + ts)
                        nc.sync.dma_start(y_out[r0:r0 + 128, :], y_sb[:])
    nc.compile()
    return nc


def _make_runner(nc, devices):
    """Jitted shard_map runner for one program over a 4-device mesh.

    Mirrors bass2jax.run_bass_via_pjrt's multi-core branch, but with an
    explicit device list so two programs can run concurrently on
    disjoint meshes.
    """
    import jax
    import concourse.mybir as mybir
    from concourse.bass2jax import _bass_exec_p, install_neuronx_cc_hook
    from jax.experimental.shard_map import shard_map
    from jax.sharding import Mesh, PartitionSpec

    from concourse.bass2jax import partition_id_tensor

    install_neuronx_cc_hook()

    partition_name = (nc.partition_id_tensor.name
                      if nc.partition_id_tensor else None)
    in_names, out_names, out_avals, zero_outs = [], [], [], []
    for alloc in nc.m.functions[0].allocations:
        if not isinstance(alloc, mybir.MemoryLocationSet):
            continue
        name = alloc.memorylocations[0].name
        if alloc.kind == "ExternalInput":
            if name != partition_name:
                in_names.append(name)
        elif alloc.kind == "ExternalOutput":
            out_names.append(name)
            shape = tuple(alloc.tensor_shape)
            dtype = mybir.dt.np(alloc.dtype)
            out_avals.append(jax.core.ShapedArray(shape, dtype))
            zero_outs.append(np.zeros(shape, dtype))
    n_params = len(in_names)
    n_outs = len(out_avals)
    all_names = in_names + out_names
    if partition_name is not None:
        all_names = all_names + [partition_name]
    donate = tuple(range(n_params, n_params + n_outs))
    n_cores = len(devices)

    def _body(*args):
        operands = list(args)
        if partition_name is not None:
            operands.append(partition_id_tensor())
        outs = _bass_exec_p.bind(
            *operands,
            out_avals=tuple(out_avals),
            in_names=tuple(all_names),
            out_names=tuple(out_names),
            lowering_input_output_aliases=(),
            sim_require_finite=True,
            sim_require_nnan=True,
            nc=nc,
        )
        return tuple(outs)

    mesh = Mesh(np.asarray(devices), ("core",))
    in_specs = (PartitionSpec("core"),) * (n_params + n_outs)
    out_specs = (PartitionSpec("core"),) * n_outs
    sharded = jax.jit(
        shard_map(_body, mesh=mesh, in_specs=in_specs, out_specs=out_specs,
                  check_rep=False),
        donate_argnums=donate, keep_unused=True)

    def run(in_maps):
        per_core = [[np.asarray(m[name]) for name in in_names] for m in in_maps]
        concat_in = [
            np.concatenate([per_core[c][i] for c in range(n_cores)], axis=0)
            for i in range(n_params)
        ]
        concat_zeros = [
            np.zeros((n_cores * z.shape[0], *z.shape[1:]), z.dtype)
            for z in zero_outs
        ]
        return sharded(*concat_in, *concat_zeros)  # async jax arrays

    def gather(out_arrs):
        return [
            {name: np.asarray(out_arrs[i]).reshape(n_cores, *out_avals[i].shape)[c]
             for i, name in enumerate(out_names)}
            for c in range(n_cores)
        ]

    return run, gather, out_names


def _tiles_for(group_starts):
    tiles = []
    for a in group_starts:
        tiles.extend(range(a, a + 4))
    return tiles


def _get_runners():
    if "runA" not in _CACHE:
        import jax
        devs = jax.devices()
        ncA = _build(GROUPS_A, KV_CHUNKS_A, Q_CHUNKS_A)
        ncB = _build(GROUPS_B, KV_CHUNKS_B, Q_CHUNKS_B)
        _CACHE["runA"] = _make_runner(ncA, devs[0:4])
        _CACHE["runB"] = _make_runner(ncB, devs[4:8])
    return _CACHE["runA"], _CACHE["runB"]


def kernel(x, Wqkv, Wproj, _trace_ctx=None):
    x = np.ascontiguousarray(x, dtype=np.float32)
    Wqkv = np.ascontiguousarray(Wqkv, dtype=np.float32)
    Wproj = np.ascontiguousarray(Wproj, dtype=np.float32)

    (runA, gatherA, _), (runB, gatherB, _) = _get_runners()

    maps = [{"x_in": x[b], "wqkv": Wqkv, "wproj": Wproj} for b in range(B)]

    import contextlib
    ctx = _trace_ctx if _trace_ctx is not None else contextlib.nullcontext()
    with ctx:
        outA = runA(maps)
        outB = runB(maps)
        resA = gatherA(outA)
        resB = gatherB(outB)

    tilesA = _tiles_for(GROUPS_A)
    tilesB = _tiles_for(GROUPS_B)
    out = np.empty((B, T, C), dtype=np.float32)
    for b in range(B):
        for slot, tile_i in enumerate(tilesA):
            out[b, 128 * tile_i:128 * (tile_i + 1)] = \
                resA[b]["y"][128 * slot:128 * (slot + 1)]
        for slot, tile_i in enumerate(tilesB):
            out[b, 128 * tile_i:128 * (tile_i + 1)] = \
                resB[b]["y"][128 * slot:128 * (slot + 1)]
    return out


# revision 5
# speedup vs baseline: 1.0842x; 1.0842x over previous
"""Self-contained Bass/Trainium2 kernel for single-head causal self-attention.

reference semantics (fp32):
  qkv = x @ Wqkv; q,k,v = split(qkv)
  att = softmax(causal(q k^T / sqrt(C)))
  y = (att @ v) @ Wproj

Sharding: 8 cores = 4 batches x 2 causally-balanced query-tile sets.
Program A (cores 0-3): q-tiles {0..7, 24..31} of its batch.
Program B (cores 4-7): q-tiles {8..23} of its batch.
Both process 72 key-chunks of attention work; each runs as its own NEFF
on a disjoint 4-device mesh, dispatched concurrently.

v2 layout: all matmul operands bf16 (halves PE weight-load time vs
f32r); V kept in SBUF (no DRAM round-trip); attention output computed
transposed (O^T = V^T-slices @ P^T) so the Wproj matmul needs no
output transposes and the softmax 1/l folds into the per-partition
PSUM eviction scale of y. Row sums l via a ones[128,128] stationary.
exp is biased by -1.5 (cancels in normalization) to center p in bf16.
The S(t+1) matmul chain is emitted before O(t) so the PE never waits
on the Act engine's exp.
"""

import sys

sys.path.insert(0, "/opt/trn_rl_repo")

import numpy as np

B, T, C = 4, 4096, 512
TQ = 2048               # q rows per core
N_CORES = 8
SCALE = 1.0 / np.sqrt(C)
MASKVAL = -1.0e10
EXP_BIAS = -1.5

GROUPS_A = [0, 4, 24, 28]    # group base tile (tiles a..a+3), program A
GROUPS_B = [8, 12, 16, 20]
KV_CHUNKS_A = 8              # 512-row x chunks needed for K/V
KV_CHUNKS_B = 6
Q_CHUNKS_A = [0, 1, 6, 7]    # x chunks holding the program's q rows
Q_CHUNKS_B = [2, 3, 4, 5]

_CACHE = {}


def _dmask_np():
    # [128, 4*512] additive masks for the 4 diagonal-offset variants.
    # Variant d, sub-tile k columns: k<d fully masked, k==d triangular
    # (valid where j' <= i'), k>d fully visible. Applied to S^T tiles
    # with keys on partitions, queries on the free dim.
    m = np.zeros((128, 4, 4, 128), dtype=np.float32)
    jj = np.arange(128)[:, None]
    ii = np.arange(128)[None, :]
    tri = np.where(jj <= ii, 0.0, MASKVAL).astype(np.float32)
    for d in range(4):
        for k in range(4):
            if k < d:
                m[:, d, k, :] = MASKVAL
            elif k == d:
                m[:, d, k, :] = tri
    return m.reshape(128, 4 * 512)


def _build(group_starts, kv_chunks, q_chunks):
    import ml_dtypes
    import concourse.mybir as mybir
    import concourse.tile as tile
    from concourse import bacc

    F32 = mybir.dt.float32
    BF16 = mybir.dt.bfloat16
    AF = mybir.ActivationFunctionType
    TKV = kv_chunks * 512
    NKEY = kv_chunks * 4         # 128-row key sub-chunks

    nc = bacc.Bacc("TRN2", target_bir_lowering=False, debug=False,
                   num_devices=4)

    x_in = nc.dram_tensor("x_in", [T, C], F32, kind="ExternalInput").ap()
    wqkv_in = nc.dram_tensor("wqkv", [C, 3 * C], F32, kind="ExternalInput").ap()
    wproj_in = nc.dram_tensor("wproj", [C, C], F32, kind="ExternalInput").ap()
    y_out = nc.dram_tensor("y", [TQ, C], F32, kind="ExternalOutput").ap()

    dmask_d = nc.inline_tensor(
        _dmask_np().astype(ml_dtypes.bfloat16), name="dmask").ap()
    ident_d = nc.inline_tensor(
        np.eye(128, dtype=ml_dtypes.bfloat16), name="ident").ap()

    with tile.TileContext(nc) as tc:
        with tc.tile_pool(name="persist", bufs=1) as pp:
            kT = pp.tile([128, 4, TKV], BF16)        # K^T  [c-chunk, j]
            qT = pp.tile([128, 4, TQ], BF16)         # Q^T  [c-chunk, i]
            v_sb = pp.tile([128, NKEY, C], BF16)     # V    [key-sub, c]
            wqkv_sb = pp.tile([128, 4, 3 * C], BF16)
            wproj_sb = pp.tile([128, 4, C], BF16)
            dm_sb = pp.tile([128, 4, 512], BF16)     # diagonal masks
            id_sb = pp.tile([128, 128], BF16)
            ones_sb = pp.tile([128, 128], BF16)
            bias_sb = pp.tile([128, 1], F32)

            nc.sync.dma_start(id_sb[:], ident_d[:])
            nc.sync.dma_start(dm_sb[:], dmask_d.rearrange("p (d n) -> p d n", d=4))
            nc.vector.memset(ones_sb[:], 1.0)
            nc.vector.memset(bias_sb[:], EXP_BIAS)
            # gpsimd casts are ~37 G elem/s — only wproj (not needed until
            # the projection tail) goes there; wqkv splits across DVE/Act.
            with tc.tile_pool(name="wtmp", bufs=2) as wt:
                for c in range(4):
                    wq_raw = wt.tile([128, 3 * C], F32, tag="wqr")
                    nc.sync.dma_start(
                        wq_raw[:], wqkv_in[128 * c:128 * (c + 1), :])
                    if c % 2 == 0:
                        nc.vector.tensor_copy(wqkv_sb[:, c, :], wq_raw[:])
                    else:
                        nc.scalar.copy(wqkv_sb[:, c, :], wq_raw[:])
                wp_raw = wt.tile([128, 4, C], F32, tag="wpr")
                nc.sync.dma_start(wp_raw[:],
                                  wproj_in.rearrange("(k p) f -> p k f", p=128))
                nc.gpsimd.tensor_copy(wproj_sb[:], wp_raw[:])

            # ---------------- Phase 1: x^T, K^T, Q^T, V ----------------
            # Software-pipelined: transposes of chunk t overlap the
            # K/Q/V matmuls of chunk t-1 (PE never waits on DVE evicts).
            with tc.tile_pool(name="p1", bufs=3) as p1, \
                 tc.tile_pool(name="p1b", bufs=2) as p1b, \
                 tc.tile_pool(name="p1xt", bufs=2, space="PSUM") as p1xt, \
                 tc.tile_pool(name="p1kv", bufs=2, space="PSUM") as p1kv:

                x_f = {}
                x_b = {}
                xT = {}

                def stage_transpose(tch):
                    x_f[tch] = p1.tile([128, 4, 512], F32, tag="x",
                                       name=f"x_f{tch}")
                    nc.sync.dma_start(
                        x_f[tch][:],
                        x_in[512 * tch:512 * (tch + 1), :]
                        .rearrange("(n p) c -> p n c", p=128))
                    x_b[tch] = p1b.tile([128, 4, 512], BF16, tag="xb",
                                        name=f"x_b{tch}")
                    if tch % 2 == 0:
                        nc.vector.tensor_copy(x_b[tch][:], x_f[tch][:])
                    else:
                        nc.scalar.copy(x_b[tch][:], x_f[tch][:])
                    xT[tch] = p1b.tile([128, 4, 512], BF16, tag="xT",
                                       name=f"xT{tch}")
                    for n in range(4):
                        ps_xt = p1xt.tile([128, 512], BF16, tag="xt")
                        for c in range(4):
                            nc.tensor.transpose(
                                ps_xt[:, 128 * c:128 * (c + 1)],
                                x_b[tch][:, n, 128 * c:128 * (c + 1)],
                                id_sb[:])
                        nc.vector.tensor_copy(
                            xT[tch][:, :, 128 * n:128 * (n + 1)],
                            ps_xt[:].rearrange("p (c q) -> p c q", c=4))

                def stage_kqv(tch):
                    xt = xT[tch]
                    # K^T tiles (evict on Act: idle during phase 1)
                    for f in range(4):
                        ps_k = p1kv.tile([128, 512], F32, tag="kv")
                        for c in range(4):
                            nc.tensor.matmul(
                                ps_k[:],
                                wqkv_sb[:, c, C + 128 * f:C + 128 * (f + 1)],
                                xt[:, c, :],
                                start=(c == 0), stop=(c == 3))
                        nc.scalar.copy(
                            kT[:, f, 512 * tch:512 * (tch + 1)], ps_k[:])
                    # V tiles -> SBUF (keys on partitions, natural layout)
                    for n in range(4):
                        ps_v = p1kv.tile([128, 512], F32, tag="kv")
                        for c in range(4):
                            nc.tensor.matmul(
                                ps_v[:],
                                xt[:, c, 128 * n:128 * (n + 1)],
                                wqkv_sb[:, c, 2 * C:3 * C],
                                start=(c == 0), stop=(c == 3))
                        nc.vector.tensor_copy(v_sb[:, 4 * tch + n, :], ps_v[:])
                    # Q^T tiles for this program's q rows
                    if tch in q_chunks:
                        slot = q_chunks.index(tch)
                        for f in range(4):
                            ps_q = p1kv.tile([128, 512], F32, tag="kv")
                            for c in range(4):
                                nc.tensor.matmul(
                                    ps_q[:],
                                    wqkv_sb[:, c, 128 * f:128 * (f + 1)],
                                    xt[:, c, :],
                                    start=(c == 0), stop=(c == 3))
                            nc.scalar.copy(
                                qT[:, f, 512 * slot:512 * (slot + 1)],
                                ps_q[:])

                for tch in range(kv_chunks + 1):
                    if tch < kv_chunks:
                        stage_transpose(tch)
                    if tch > 0:
                        stage_kqv(tch - 1)
                        del x_f[tch - 1], x_b[tch - 1]

            # ---------------- Phase 2: attention + projection ----------
            with tc.tile_pool(name="p2", bufs=1) as p2, \
                 tc.tile_pool(name="psS", bufs=2, space="PSUM") as psS, \
                 tc.tile_pool(name="psO", bufs=1, space="PSUM") as psO, \
                 tc.tile_pool(name="psl", bufs=1, space="PSUM") as psl:

                def s_chunk(g, a, t):
                    s_ps = psS.tile([128, 512], F32, tag="s", name=f"s{g}_{t}")
                    for c in range(4):
                        nc.tensor.matmul(
                            s_ps[:],
                            kT[:, c, 128 * t:128 * (t + 1)],
                            qT[:, c, 512 * g:512 * (g + 1)],
                            start=(c == 0), stop=(c == 3))
                    d = t - a
                    if d >= 0:
                        nc.vector.tensor_add(s_ps[:], s_ps[:], dm_sb[:, d, :])
                    pT = p2.tile([128, 512], BF16, tag="pT", bufs=4)
                    nc.scalar.activation(pT[:], s_ps[:], AF.Exp,
                                         bias=bias_sb[:], scale=SCALE)
                    return pT

                def o_chunk(o_ps, l_ps, trip, t, pT):
                    first, last = (t == 0), (t == trip - 1)
                    for cs in range(4):
                        nc.tensor.matmul(
                            o_ps[cs][:],
                            v_sb[:, t, 128 * cs:128 * (cs + 1)],
                            pT[:],
                            start=first, stop=last,
                            skip_group_check=True)
                    nc.tensor.matmul(
                        l_ps[:], ones_sb[:], pT[:],
                        start=first, stop=last,
                        skip_group_check=True)

                def make_tail(g, o_ps, l_ps):
                    def tail():
                        # normalization scalars: r = 1/l per query row.
                        # lT borrows the o0 bank between o and y uses.
                        l_sb = p2.tile([128, 512], BF16, tag="lsb", bufs=2)
                        nc.vector.tensor_copy(l_sb[:], l_ps[:])
                        lT_ps = psO.tile([128, 4, 128], BF16, tag="o0",
                                         name=f"lT{g}")
                        for ts in range(4):
                            nc.tensor.transpose(
                                lT_ps[:, ts, :],
                                l_sb[:, 128 * ts:128 * (ts + 1)], id_sb[:])
                        r_sb = p2.tile([128, 4], F32, tag="r", bufs=2)
                        for ts in range(4):
                            nc.vector.reciprocal(r_sb[:, ts:ts + 1],
                                                 lT_ps[:, ts, 0:1])

                        # attention output (transposed) -> bf16
                        oT_sb = p2.tile([128, 4, 512], BF16, tag="oT", bufs=2)
                        for cs in range(4):
                            nc.vector.tensor_copy(oT_sb[:, cs, :], o_ps[cs][:])

                        # projection: y[tok, f] over c-subtiles, scaled by
                        # r at eviction (each ts reuses an o bank)
                        for ts in range(4):
                            y_ps = psO.tile([128, 512], F32, tag=f"o{ts}",
                                            name=f"y{g}_{ts}")
                            for c in range(4):
                                nc.tensor.matmul(
                                    y_ps[:],
                                    oT_sb[:, c, 128 * ts:128 * (ts + 1)],
                                    wproj_sb[:, c, :],
                                    start=(c == 0), stop=(c == 3))
                            y_sb = p2.tile([128, 512], F32, tag="ysb", bufs=2)
                            nc.vector.tensor_scalar_mul(y_sb[:], y_ps[:],
                                                        r_sb[:, ts:ts + 1])
                            r0 = 128 * (4 * g + ts)
                            nc.sync.dma_start(y_out[r0:r0 + 128, :], y_sb[:])
                    return tail

                # two-level software pipeline: within a group S(t+1) is
                # emitted before O(t) (PE never waits on exp); across
                # groups the first two S tiles of g+1 are emitted before
                # g's projection tail so its evict latency is covered.
                tail_prev = None
                for g, a in enumerate(group_starts):
                    trip = a + 4
                    pT0 = s_chunk(g, a, 0)
                    pT1 = s_chunk(g, a, 1)
                    if tail_prev is not None:
                        tail_prev()
                    o_ps = [psO.tile([128, 512], F32, tag=f"o{cs}",
                                     name=f"o_ps{g}_{cs}") for cs in range(4)]
                    l_ps = psl.tile([128, 512], F32, tag="l", name=f"l{g}")
                    o_chunk(o_ps, l_ps, trip, 0, pT0)
                    pT_prev = pT1
                    for t in range(2, trip):
                        pT_cur = s_chunk(g, a, t)
                        o_chunk(o_ps, l_ps, trip, t - 1, pT_prev)
                        pT_prev = pT_cur
                    o_chunk(o_ps, l_ps, trip, trip - 1, pT_prev)
                    tail_prev = make_tail(g, o_ps, l_ps)
                tail_prev()
    nc.compile()
    return nc


def _make_runner(nc, devices):
    """Jitted shard_map runner for one program over a 4-device mesh.

    Mirrors bass2jax.run_bass_via_pjrt's multi-core branch, but with an
    explicit device list so two programs can run concurrently on
    disjoint meshes.
    """
    import jax
    import concourse.mybir as mybir
    from concourse.bass2jax import _bass_exec_p, install_neuronx_cc_hook
    from jax.experimental.shard_map import shard_map
    from jax.sharding import Mesh, PartitionSpec

    from concourse.bass2jax import partition_id_tensor

    install_neuronx_cc_hook()

    partition_name = (nc.partition_id_tensor.name
                      if nc.partition_id_tensor else None)
    in_names, out_names, out_avals, zero_outs = [], [], [], []
    for alloc in nc.m.functions[0].allocations:
        if not isinstance(alloc, mybir.MemoryLocationSet):
            continue
        name = alloc.memorylocations[0].name
        if alloc.kind == "ExternalInput":
            if name != partition_name:
                in_names.append(name)
        elif alloc.kind == "ExternalOutput":
            out_names.append(name)
            shape = tuple(alloc.tensor_shape)
            dtype = mybir.dt.np(alloc.dtype)
            out_avals.append(jax.core.ShapedArray(shape, dtype))
            zero_outs.append(np.zeros(shape, dtype))
    n_params = len(in_names)
    n_outs = len(out_avals)
    all_names = in_names + out_names
    if partition_name is not None:
        all_names = all_names + [partition_name]
    donate = tuple(range(n_params, n_params + n_outs))
    n_cores = len(devices)

    def _body(*args):
        operands = list(args)
        if partition_name is not None:
            operands.append(partition_id_tensor())
        outs = _bass_exec_p.bind(
            *operands,
            out_avals=tuple(out_avals),
            in_names=tuple(all_names),
            out_names=tuple(out_names),
            lowering_input_output_aliases=(),
            sim_require_finite=True,
            sim_require_nnan=True,
            nc=nc,
        )
        return tuple(outs)

    mesh = Mesh(np.asarray(devices), ("core",))
    in_specs = (PartitionSpec("core"),) * (n_params + n_outs)
    out_specs = (PartitionSpec("core"),) * n_outs
    sharded = jax.jit(
        shard_map(_body, mesh=mesh, in_specs=in_specs, out_specs=out_specs,
                  check_rep=False),
        donate_argnums=donate, keep_unused=True)

    def run(in_maps):
        per_core = [[np.asarray(m[name]) for name in in_names] for m in in_maps]
        concat_in = [
            np.concatenate([per_core[c][i] for c in range(n_cores)], axis=0)
            for i in range(n_params)
        ]
        concat_zeros = [
            np.zeros((n_cores * z.shape[0], *z.shape[1:]), z.dtype)
            for z in zero_outs
        ]
        return sharded(*concat_in, *concat_zeros)  # async jax arrays

    def gather(out_arrs):
        return [
            {name: np.asarray(out_arrs[i]).reshape(n_cores, *out_avals[i].shape)[c]
             for i, name in enumerate(out_names)}
            for c in range(n_cores)
        ]

    return run, gather, out_names


def _tiles_for(group_starts):
    tiles = []
    for a in group_starts:
        tiles.extend(range(a, a + 4))
    return tiles


def _get_runners():
    if "runA" not in _CACHE:
        import jax
        devs = jax.devices()
        ncA = _build(GROUPS_A, KV_CHUNKS_A, Q_CHUNKS_A)
        ncB = _build(GROUPS_B, KV_CHUNKS_B, Q_CHUNKS_B)
        _CACHE["runA"] = _make_runner(ncA, devs[0:4])
        _CACHE["runB"] = _make_runner(ncB, devs[4:8])
    return _CACHE["runA"], _CACHE["runB"]


def kernel(x, Wqkv, Wproj, _trace_ctx=None):
    x = np.ascontiguousarray(x, dtype=np.float32)
    Wqkv = np.ascontiguousarray(Wqkv, dtype=np.float32)
    Wproj = np.ascontiguousarray(Wproj, dtype=np.float32)

    (runA, gatherA, _), (runB, gatherB, _) = _get_runners()

    maps = [{"x_in": x[b], "wqkv": Wqkv, "wproj": Wproj} for b in range(B)]

    import contextlib
    ctx = _trace_ctx if _trace_ctx is not None else contextlib.nullcontext()
    with ctx:
        outA = runA(maps)
        outB = runB(maps)
        resA = gatherA(outA)
        resB = gatherB(outB)

    tilesA = _tiles_for(GROUPS_A)
    tilesB = _tiles_for(GROUPS_B)
    out = np.empty((B, T, C), dtype=np.float32)
    for b in range(B):
        for slot, tile_i in enumerate(tilesA):
            out[b, 128 * tile_i:128 * (tile_i + 1)] = \
                resA[b]["y"][128 * slot:128 * (slot + 1)]
        for slot, tile_i in enumerate(tilesB):
            out[b, 128 * tile_i:128 * (tile_i + 1)] = \
                resB[b]["y"][128 * slot:128 * (slot + 1)]
    return out


# revision 8
# speedup vs baseline: 1.1385x; 1.0501x over previous
"""Self-contained Bass/Trainium2 kernel for single-head causal self-attention.

reference semantics (fp32):
  qkv = x @ Wqkv; q,k,v = split(qkv)
  att = softmax(causal(q k^T / sqrt(C)))
  y = (att @ v) @ Wproj

Sharding: 8 cores = 4 batches x 2 causally-balanced query-tile sets.
Program A (cores 0-3): q-tiles {0..7, 24..31} of its batch.
Program B (cores 4-7): q-tiles {8..23} of its batch.
Both process 72 key-chunks of attention work; each runs as its own NEFF
on a disjoint 4-device mesh, dispatched concurrently.

v2 layout: all matmul operands bf16 (halves PE weight-load time vs
f32r); V kept in SBUF (no DRAM round-trip); attention output computed
transposed (O^T = V^T-slices @ P^T) so the Wproj matmul needs no
output transposes and the softmax 1/l folds into the per-partition
PSUM eviction scale of y. Row sums l via a ones[128,128] stationary.
exp is biased by -1.5 (cancels in normalization) to center p in bf16.
The S(t+1) matmul chain is emitted before O(t) so the PE never waits
on the Act engine's exp.
"""

import sys

sys.path.insert(0, "/opt/trn_rl_repo")

import numpy as np

B, T, C = 4, 4096, 512
TQ = 2048               # q rows per core
N_CORES = 8
SCALE = 1.0 / np.sqrt(C)
MASKVAL = -1.0e10
EXP_BIAS = -1.5

GROUPS_A = [0, 4, 24, 28]    # group base tile (tiles a..a+3), program A
GROUPS_B = [8, 12, 16, 20]
KV_CHUNKS_A = 8              # 512-row x chunks needed for K/V
KV_CHUNKS_B = 6
Q_CHUNKS_A = [0, 1, 6, 7]    # x chunks holding the program's q rows
Q_CHUNKS_B = [2, 3, 4, 5]

_CACHE = {}


def _dmask_np():
    # [128, 4*512] additive masks for the 4 diagonal-offset variants.
    # Variant d, sub-tile k columns: k<d fully masked, k==d triangular
    # (valid where j' <= i'), k>d fully visible. Applied to S^T tiles
    # with keys on partitions, queries on the free dim.
    m = np.zeros((128, 4, 4, 128), dtype=np.float32)
    jj = np.arange(128)[:, None]
    ii = np.arange(128)[None, :]
    tri = np.where(jj <= ii, 0.0, MASKVAL).astype(np.float32)
    for d in range(4):
        for k in range(4):
            if k < d:
                m[:, d, k, :] = MASKVAL
            elif k == d:
                m[:, d, k, :] = tri
    return m.reshape(128, 4 * 512)


def _build(group_starts, kv_chunks, q_chunks):
    import ml_dtypes
    import concourse.mybir as mybir
    import concourse.tile as tile
    from concourse import bacc

    F32 = mybir.dt.float32
    BF16 = mybir.dt.bfloat16
    AF = mybir.ActivationFunctionType
    TKV = kv_chunks * 512
    NKEY = kv_chunks * 4         # 128-row key sub-chunks

    nc = bacc.Bacc("TRN2", target_bir_lowering=False, debug=False,
                   num_devices=4)

    x_in = nc.dram_tensor("x_in", [T, C], F32, kind="ExternalInput").ap()
    wqkv_in = nc.dram_tensor("wqkv", [C, 3 * C], F32, kind="ExternalInput").ap()
    wproj_in = nc.dram_tensor("wproj", [C, C], F32, kind="ExternalInput").ap()
    y_out = nc.dram_tensor("y", [TQ, C], F32, kind="ExternalOutput").ap()

    dmask_d = nc.inline_tensor(
        _dmask_np().astype(ml_dtypes.bfloat16), name="dmask").ap()
    ident_d = nc.inline_tensor(
        np.eye(128, dtype=ml_dtypes.bfloat16), name="ident").ap()

    with tile.TileContext(nc) as tc:
        with tc.tile_pool(name="persist", bufs=1) as pp:
            kT = pp.tile([128, 4, TKV], BF16)        # K^T  [c-chunk, j]
            qT = pp.tile([128, 4, TQ], BF16)         # Q^T  [c-chunk, i]
            v_sb = pp.tile([128, NKEY, C], BF16)     # V    [key-sub, c]
            wqkv_sb = pp.tile([128, 4, 3 * C], BF16)
            wproj_sb = pp.tile([128, 4, C], BF16)
            dm_sb = pp.tile([128, 4, 512], BF16)     # diagonal masks
            id_sb = pp.tile([128, 128], BF16)
            ones_sb = pp.tile([128, 128], BF16)
            bias_sb = pp.tile([128, 1], F32)

            nc.sync.dma_start(id_sb[:], ident_d[:])
            nc.vector.memset(ones_sb[:], 1.0)
            nc.vector.memset(bias_sb[:], EXP_BIAS)

            # ---------------- Phase 1: x^T, K^T, Q^T, V ----------------
            # Software-pipelined: transposes of chunk t overlap the
            # K/Q/V matmuls of chunk t-1 (PE never waits on DVE evicts).
            with tc.tile_pool(name="p1", bufs=4) as p1, \
                 tc.tile_pool(name="p1b", bufs=2) as p1b, \
                 tc.tile_pool(name="p1xt", bufs=2, space="PSUM") as p1xt, \
                 tc.tile_pool(name="p1kv", bufs=2, space="PSUM") as p1kv:

                x_f = {}
                x_b = {}
                xT = {}

                def stage_transpose(tch):
                    x_f[tch] = p1.tile([128, 4, 512], F32, tag="x",
                                       name=f"x_f{tch}")
                    nc.sync.dma_start(
                        x_f[tch][:],
                        x_in[512 * tch:512 * (tch + 1), :]
                        .rearrange("(n p) c -> p n c", p=128))
                    x_b[tch] = p1b.tile([128, 4, 512], BF16, tag="xb",
                                        name=f"x_b{tch}")
                    xT[tch] = p1b.tile([128, 4, 512], BF16, tag="xT",
                                       name=f"xT{tch}")
                    for n in range(4):
                        # per-subtile casts so the first transpose starts
                        # as soon as a quarter of the chunk is converted
                        if (tch + n) % 2 == 0:
                            nc.vector.tensor_copy(x_b[tch][:, n, :],
                                                  x_f[tch][:, n, :])
                        else:
                            nc.scalar.copy(x_b[tch][:, n, :],
                                           x_f[tch][:, n, :])
                        ps_xt = p1xt.tile([128, 512], BF16, tag="xt")
                        for c in range(4):
                            nc.tensor.transpose(
                                ps_xt[:, 128 * c:128 * (c + 1)],
                                x_b[tch][:, n, 128 * c:128 * (c + 1)],
                                id_sb[:])
                        nc.vector.tensor_copy(
                            xT[tch][:, :, 128 * n:128 * (n + 1)],
                            ps_xt[:].rearrange("p (c q) -> p c q", c=4))

                def stage_kqv(tch):
                    xt = xT[tch]
                    # K^T tiles (evict on Act: idle during phase 1)
                    for f in range(4):
                        ps_k = p1kv.tile([128, 512], F32, tag="kv")
                        for c in range(4):
                            nc.tensor.matmul(
                                ps_k[:],
                                wqkv_sb[:, c, C + 128 * f:C + 128 * (f + 1)],
                                xt[:, c, :],
                                start=(c == 0), stop=(c == 3))
                        nc.scalar.copy(
                            kT[:, f, 512 * tch:512 * (tch + 1)], ps_k[:])
                    # V tiles -> SBUF (keys on partitions, natural layout)
                    for n in range(4):
                        ps_v = p1kv.tile([128, 512], F32, tag="kv")
                        for c in range(4):
                            nc.tensor.matmul(
                                ps_v[:],
                                xt[:, c, 128 * n:128 * (n + 1)],
                                wqkv_sb[:, c, 2 * C:3 * C],
                                start=(c == 0), stop=(c == 3))
                        nc.vector.tensor_copy(v_sb[:, 4 * tch + n, :], ps_v[:])
                    # Q^T tiles for this program's q rows
                    if tch in q_chunks:
                        slot = q_chunks.index(tch)
                        for f in range(4):
                            ps_q = p1kv.tile([128, 512], F32, tag="kv")
                            for c in range(4):
                                nc.tensor.matmul(
                                    ps_q[:],
                                    wqkv_sb[:, c, 128 * f:128 * (f + 1)],
                                    xt[:, c, :],
                                    start=(c == 0), stop=(c == 3))
                            nc.scalar.copy(
                                qT[:, f, 512 * slot:512 * (slot + 1)],
                                ps_q[:])

                # x(0) DMA first so the PE starts transposing at ~4us;
                # weight DMAs follow and the first K accumulation chain
                # consumes the wqkv planes as they arrive.
                stage_transpose(0)
                with tc.tile_pool(name="wtmp", bufs=2) as wt:
                    for c in range(4):
                        wq_raw = wt.tile([128, 3 * C], F32, tag="wqr")
                        nc.sync.dma_start(
                            wq_raw[:], wqkv_in[128 * c:128 * (c + 1), :])
                        if c % 2 == 0:
                            nc.vector.tensor_copy(wqkv_sb[:, c, :], wq_raw[:])
                        else:
                            nc.scalar.copy(wqkv_sb[:, c, :], wq_raw[:])
                    wp_raw = wt.tile([128, 4, C], F32, tag="wpr")
                    nc.sync.dma_start(
                        wp_raw[:],
                        wproj_in.rearrange("(k p) f -> p k f", p=128))
                    nc.gpsimd.tensor_copy(wproj_sb[:], wp_raw[:])
                nc.sync.dma_start(
                    dm_sb[:], dmask_d.rearrange("p (d n) -> p d n", d=4))

                for tch in range(1, kv_chunks + 1):
                    if tch < kv_chunks:
                        stage_transpose(tch)
                    stage_kqv(tch - 1)
                    del x_f[tch - 1], x_b[tch - 1]

            # ---------------- Phase 2: attention + projection ----------
            with tc.tile_pool(name="p2", bufs=1) as p2, \
                 tc.tile_pool(name="psS", bufs=2, space="PSUM") as psS, \
                 tc.tile_pool(name="psO", bufs=1, space="PSUM") as psO, \
                 tc.tile_pool(name="psl", bufs=1, space="PSUM") as psl:

                def s_chunk(g, a, t):
                    s_ps = psS.tile([128, 512], F32, tag="s", name=f"s{g}_{t}")
                    for c in range(4):
                        nc.tensor.matmul(
                            s_ps[:],
                            kT[:, c, 128 * t:128 * (t + 1)],
                            qT[:, c, 512 * g:512 * (g + 1)],
                            start=(c == 0), stop=(c == 3))
                    d = t - a
                    if d >= 0:
                        nc.vector.tensor_add(s_ps[:], s_ps[:], dm_sb[:, d, :])
                    pT = p2.tile([128, 512], BF16, tag="pT", bufs=4)
                    nc.scalar.activation(pT[:], s_ps[:], AF.Exp,
                                         bias=bias_sb[:], scale=SCALE)
                    return pT

                def o_chunk(o_ps, l_ps, trip, t, pT):
                    first, last = (t == 0), (t == trip - 1)
                    for cs in range(4):
                        nc.tensor.matmul(
                            o_ps[cs][:],
                            v_sb[:, t, 128 * cs:128 * (cs + 1)],
                            pT[:],
                            start=first, stop=last,
                            skip_group_check=True)
                    nc.tensor.matmul(
                        l_ps[:], ones_sb[:], pT[:],
                        start=first, stop=last,
                        skip_group_check=True)

                def make_tail(g, o_ps, l_ps):
                    def tail():
                        # normalization scalars: r = 1/l per query row.
                        # lT borrows the o0 bank between o and y uses.
                        l_sb = p2.tile([128, 512], BF16, tag="lsb", bufs=2)
                        nc.scalar.copy(l_sb[:], l_ps[:])
                        lT_ps = psO.tile([128, 4, 128], BF16, tag="o0",
                                         name=f"lT{g}")
                        for ts in range(4):
                            nc.tensor.transpose(
                                lT_ps[:, ts, :],
                                l_sb[:, 128 * ts:128 * (ts + 1)], id_sb[:])
                        r_sb = p2.tile([128, 4], F32, tag="r", bufs=2)
                        for ts in range(4):
                            nc.vector.reciprocal(r_sb[:, ts:ts + 1],
                                                 lT_ps[:, ts, 0:1])

                        # attention output (transposed) -> bf16
                        oT_sb = p2.tile([128, 4, 512], BF16, tag="oT", bufs=2)
                        for cs in range(4):
                            if cs % 2 == 0:
                                nc.vector.tensor_copy(oT_sb[:, cs, :],
                                                      o_ps[cs][:])
                            else:
                                nc.scalar.copy(oT_sb[:, cs, :], o_ps[cs][:])

                        # projection: y[tok, f] over c-subtiles, scaled by
                        # r at eviction (each ts reuses an o bank)
                        for ts in range(4):
                            y_ps = psO.tile([128, 512], F32, tag=f"o{ts}",
                                            name=f"y{g}_{ts}")
                            for c in range(4):
                                nc.tensor.matmul(
                                    y_ps[:],
                                    oT_sb[:, c, 128 * ts:128 * (ts + 1)],
                                    wproj_sb[:, c, :],
                                    start=(c == 0), stop=(c == 3))
                            y_sb = p2.tile([128, 512], F32, tag="ysb", bufs=2)
                            nc.vector.tensor_scalar_mul(y_sb[:], y_ps[:],
                                                        r_sb[:, ts:ts + 1])
                            r0 = 128 * (4 * g + ts)
                            nc.sync.dma_start(y_out[r0:r0 + 128, :], y_sb[:])
                    return tail

                # two-level software pipeline: within a group S(t+1) is
                # emitted before O(t) (PE never waits on exp); across
                # groups the first two S tiles of g+1 are emitted before
                # g's projection tail so its evict latency is covered.
                tail_prev = None
                for g, a in enumerate(group_starts):
                    trip = a + 4
                    pT0 = s_chunk(g, a, 0)
                    pT1 = s_chunk(g, a, 1)
                    if tail_prev is not None:
                        tail_prev()
                    o_ps = [psO.tile([128, 512], F32, tag=f"o{cs}",
                                     name=f"o_ps{g}_{cs}") for cs in range(4)]
                    l_ps = psl.tile([128, 512], F32, tag="l", name=f"l{g}")
                    o_chunk(o_ps, l_ps, trip, 0, pT0)
                    pT_prev = pT1
                    for t in range(2, trip):
                        pT_cur = s_chunk(g, a, t)
                        o_chunk(o_ps, l_ps, trip, t - 1, pT_prev)
                        pT_prev = pT_cur
                    o_chunk(o_ps, l_ps, trip, trip - 1, pT_prev)
                    tail_prev = make_tail(g, o_ps, l_ps)
                tail_prev()
    nc.compile()
    return nc


def _make_runner(nc, devices):
    """Jitted shard_map runner for one program over a 4-device mesh.

    Mirrors bass2jax.run_bass_via_pjrt's multi-core branch, but with an
    explicit device list so two programs can run concurrently on
    disjoint meshes.
    """
    import jax
    import concourse.mybir as mybir
    from concourse.bass2jax import _bass_exec_p, install_neuronx_cc_hook
    from jax.experimental.shard_map import shard_map
    from jax.sharding import Mesh, PartitionSpec

    from concourse.bass2jax import partition_id_tensor

    install_neuronx_cc_hook()

    partition_name = (nc.partition_id_tensor.name
                      if nc.partition_id_tensor else None)
    in_names, out_names, out_avals, zero_outs = [], [], [], []
    for alloc in nc.m.functions[0].allocations:
        if not isinstance(alloc, mybir.MemoryLocationSet):
            continue
        name = alloc.memorylocations[0].name
        if alloc.kind == "ExternalInput":
            if name != partition_name:
                in_names.append(name)
        elif alloc.kind == "ExternalOutput":
            out_names.append(name)
            shape = tuple(alloc.tensor_shape)
            dtype = mybir.dt.np(alloc.dtype)
            out_avals.append(jax.core.ShapedArray(shape, dtype))
            zero_outs.append(np.zeros(shape, dtype))
    n_params = len(in_names)
    n_outs = len(out_avals)
    all_names = in_names + out_names
    if partition_name is not None:
        all_names = all_names + [partition_name]
    donate = tuple(range(n_params, n_params + n_outs))
    n_cores = len(devices)

    def _body(*args):
        operands = list(args)
        if partition_name is not None:
            operands.append(partition_id_tensor())
        outs = _bass_exec_p.bind(
            *operands,
            out_avals=tuple(out_avals),
            in_names=tuple(all_names),
            out_names=tuple(out_names),
            lowering_input_output_aliases=(),
            sim_require_finite=True,
            sim_require_nnan=True,
            nc=nc,
        )
        return tuple(outs)

    mesh = Mesh(np.asarray(devices), ("core",))
    in_specs = (PartitionSpec("core"),) * (n_params + n_outs)
    out_specs = (PartitionSpec("core"),) * n_outs
    sharded = jax.jit(
        shard_map(_body, mesh=mesh, in_specs=in_specs, out_specs=out_specs,
                  check_rep=False),
        donate_argnums=donate, keep_unused=True)

    def run(in_maps):
        per_core = [[np.asarray(m[name]) for name in in_names] for m in in_maps]
        concat_in = [
            np.concatenate([per_core[c][i] for c in range(n_cores)], axis=0)
            for i in range(n_params)
        ]
        concat_zeros = [
            np.zeros((n_cores * z.shape[0], *z.shape[1:]), z.dtype)
            for z in zero_outs
        ]
        return sharded(*concat_in, *concat_zeros)  # async jax arrays

    def gather(out_arrs):
        return [
            {name: np.asarray(out_arrs[i]).reshape(n_cores, *out_avals[i].shape)[c]
             for i, name in enumerate(out_names)}
            for c in range(n_cores)
        ]

    return run, gather, out_names


def _tiles_for(group_starts):
    tiles = []
    for a in group_starts:
        tiles.extend(range(a, a + 4))
    return tiles


def _get_runners():
    if "runA" not in _CACHE:
        import jax
        devs = jax.devices()
        ncA = _build(GROUPS_A, KV_CHUNKS_A, Q_CHUNKS_A)
        ncB = _build(GROUPS_B, KV_CHUNKS_B, Q_CHUNKS_B)
        _CACHE["runA"] = _make_runner(ncA, devs[0:4])
        _CACHE["runB"] = _make_runner(ncB, devs[4:8])
    return _CACHE["runA"], _CACHE["runB"]


def kernel(x, Wqkv, Wproj, _trace_ctx=None):
    x = np.ascontiguousarray(x, dtype=np.float32)
    Wqkv = np.ascontiguousarray(Wqkv, dtype=np.float32)
    Wproj = np.ascontiguousarray(Wproj, dtype=np.float32)

    (runA, gatherA, _), (runB, gatherB, _) = _get_runners()

    maps = [{"x_in": x[b], "wqkv": Wqkv, "wproj": Wproj} for b in range(B)]

    import contextlib
    ctx = _trace_ctx if _trace_ctx is not None else contextlib.nullcontext()
    with ctx:
        outA = runA(maps)
        outB = runB(maps)
        resA = gatherA(outA)
        resB = gatherB(outB)

    tilesA = _tiles_for(GROUPS_A)
    tilesB = _tiles_for(GROUPS_B)
    out = np.empty((B, T, C), dtype=np.float32)
    for b in range(B):
        for slot, tile_i in enumerate(tilesA):
            out[b, 128 * tile_i:128 * (tile_i + 1)] = \
                resA[b]["y"][128 * slot:128 * (slot + 1)]
        for slot, tile_i in enumerate(tilesB):
            out[b, 128 * tile_i:128 * (tile_i + 1)] = \
                resB[b]["y"][128 * slot:128 * (slot + 1)]
    return out
